# revision 2
# baseline (speedup 1.0000x reference)
"""VQ-codebook linear layer on 8 Trainium2 NeuronCores.

Problem: out = x_fp16 @ W_q.T where W_q = codebook[indices].reshape(4096, 4096)
  x:        (4, 2048, 4096) fp32
  codebook: (256, 8) fp16
  indices:  (2097152,) int64
  out:      (4, 2048, 4096) fp32

Sharding: 4 m-shards x 2 o-shards. Core c = mi*2 + oj computes
  out[mi*2048:(mi+1)*2048, oj*2048:(oj+1)*2048]
  -> per-core matmul [2048, 4096] x [4096, 2048], fp16 operands, fp32 accum.

Per-core device kernel:
  - W.T shard resident in SBUF: 32 tiles [128 k, 2048 o] fp16 (16 MB)
  - per m-tile (128 rows): load x-tile [128 p, 32 kc, 128 m] (1 MB, pre-tiled
    on host so the DMA is a single contiguous run per partition), accumulate
    4 PSUM banks [128 m, 512 o] over 32 k-chunks; 4 matmuls per k-chunk share
    one stationary slice, so the serial LDWEIGHTS cost is paid once per 4
    matmuls' worth of reorder slack; the other 4 banks drain in parallel
  - drain: 4 DVE copies into one [128, 2048] SBUF tile, single 1 MB DMA out

vs. the o-column-parallel baseline (564 us): 4x less x traffic per core,
stationary shared across 4 consecutive matmuls, psum ping-pong instead of
full-bank drain stalls. Measured ~517 us steady-state per iteration.
"""

import numpy as np

import concourse.bacc as bacc
import concourse.mybir as mybir
import concourse.tile as tile
from concourse import bass_utils

B, S, K = 4, 2048, 4096          # batch, seq, in_features
M = B * S                        # 8192 tokens
O = 4096                         # out_features
CORES = 8
MSH, OSH = 4, 2                  # m-shards x o-shards
MB = M // MSH                    # 2048 m per core
OB = O // OSH                    # 2048 o per core
KC = K // 128                    # 32 k-chunks
MT = MB // 128                   # 16 m-tiles per core
NJ = OB // 512                   # 4 psum banks per m-tile

_cached = None


def _build(repeat=1, xt_bufs=3, ot_bufs=3):
    nc = bacc.Bacc("TRN2", target_bir_lowering=False, debug=False)

    xT_d = nc.dram_tensor("xTt", [MT, 128, KC, 128], mybir.dt.float16,
                          kind="ExternalInput")
    wT_d = nc.dram_tensor("wT", [K, OB], mybir.dt.float16, kind="ExternalInput")
    out_d = nc.dram_tensor("out", [MB, OB], mybir.dt.float32,
                           kind="ExternalOutput")

    with tile.TileContext(nc) as tc:
        with (
            tc.tile_pool(name="wt", bufs=1) as wt_pool,
            tc.tile_pool(name="xt", bufs=xt_bufs) as xt_pool,
            tc.tile_pool(name="ot", bufs=ot_bufs) as out_pool,
            tc.tile_pool(name="ps", bufs=8, space="PSUM") as psum_pool,
        ):
            # resident weight shard: 32 tiles [128, 2048] fp16 (16 MB)
            wt_tiles = []
            for kc in range(KC):
                wt = wt_pool.tile([128, OB], mybir.dt.float16, tag=f"wt{kc}")
                nc.sync.dma_start(out=wt[:], in_=wT_d[kc * 128:(kc + 1) * 128, :])
                wt_tiles.append(wt)

            for rep, mt in ((r, t) for r in range(repeat) for t in range(MT)):
                xt = xt_pool.tile([128, KC, 128], mybir.dt.float16, tag="xt")
                nc.sync.dma_start(out=xt[:], in_=xT_d[mt])
                psums = [
                    psum_pool.tile([128, 512], mybir.dt.float32, tag="ps",
                                   name=f"ps{rep}_{mt}_{j}")
                    for j in range(NJ)
                ]
                for kc in range(KC):
                    lhsT = xt[:, kc, :]
                    for j in range(NJ):
                        nc.tensor.matmul(
                            psums[j][:],
                            lhsT=lhsT,
                            rhs=wt_tiles[kc][:, j * 512:(j + 1) * 512],
                            start=(kc == 0),
                            stop=(kc == KC - 1),
                        )
                o_sb = out_pool.tile([128, OB], mybir.dt.float32, tag="ot",
                                     name=f"ot{rep}_{mt}")
                for j in range(NJ):
                    nc.vector.tensor_copy(out=o_sb[:, j * 512:(j + 1) * 512],
                                          in_=psums[j][:])
                nc.sync.dma_start(
                    out=out_d[mt * 128:(mt + 1) * 128, :], in_=o_sb[:]
                )

    nc.compile()
    return nc


def make_in_maps(inputs):
    """Host-side staging: shard + pre-tile the full inputs per core."""
    x = np.asarray(inputs["x"])
    codebook = np.asarray(inputs["codebook"]).astype(np.float16, copy=False)
    indices = np.asarray(inputs["indices"])

    x2 = x.reshape(M, K).astype(np.float16)
    W = codebook[indices.astype(np.int64)].reshape(O, K)  # fp16 [4096, 4096]

    x_shards = []
    for mi in range(MSH):
        xs = x2[mi * MB:(mi + 1) * MB, :]                  # [2048, 4096]
        # [mt, p, kc, m']  with value x[mt*128+m', kc*128+p]
        xt = np.ascontiguousarray(
            xs.reshape(MT, 128, KC, 128).transpose(0, 3, 2, 1)
        )
        x_shards.append(xt)
    w_shards = [
        np.ascontiguousarray(W[oj * OB:(oj + 1) * OB, :].T)   # [4096, 2048]
        for oj in range(OSH)
    ]

    in_maps = []
    for c in range(CORES):
        mi, oj = divmod(c, OSH)
        in_maps.append({"xTt": x_shards[mi], "wT": w_shards[oj]})
    return in_maps


def assemble(results):
    out = np.empty((M, O), dtype=np.float32)
    for c in range(CORES):
        mi, oj = divmod(c, OSH)
        out[mi * MB:(mi + 1) * MB, oj * OB:(oj + 1) * OB] = results[c]["out"]
    return out.reshape(B, S, O)


def kernel(x, codebook, indices):
    global _cached
    if _cached is None:
        _cached = _build()
    nc = _cached
    in_maps = make_in_maps({"x": x, "codebook": codebook, "indices": indices})
    res = bass_utils.run_bass_kernel_spmd(nc, in_maps, core_ids=list(range(CORES)))
    return assemble(res.results)
